# revision 22
# baseline (speedup 1.0000x reference)
"""Two-layer GraphConv (GCN) on 8 Trainium2 NeuronCores.

Reference computation (per layer):
    h   = x @ W                      [N, D]
    msg = h[edge_src] * edge_weight  [E, D]
    out = segment_sum(msg, edge_dst) [N, D]
    x'  = tanh(out)

Distribution strategy (from the sharding hint): partition nodes across the 8
cores by destination range (12500 nodes/core). Each core:
  - computes h for its own node slice (dense matmul, fp16),
  - AllGathers the full h table into DRAM,
  - gathers h[src] rows for its own (dst-sorted) edges with dma_gather,
  - aggregates per 128-dst block with a one-hot selector matmul into PSUM
    (sel[e, d] = ew[e] * (d == dst_off[e])); the selector matrices are
    precomputed on the host and streamed from DRAM on the HWDGE path, so no
    on-chip engine spends time building them,
  - applies tanh on the Scalar engine.

Layer 1 produces its output transposed ([feat, node]) so that layer 2's dense
matmul needs no runtime transpose; layer 2 produces [node, feat] directly for
the final output.

dma_gather uses int16 indices, so the h table is addressed in 4 row-chunks
(< 32768 rows each). Edges are bucketed by (dst block, src chunk); each
bucket is padded to a multiple of 128 slots with ew=0 entries so the matmul
schedule is static and identical across cores (SPMD: one instruction stream).
The schedule constants are derived from the actual edge data at call time,
then compiled fresh — so the kernel is correct for any input values.
"""

import numpy as np

import concourse.bacc as bacc
import concourse.mybir as mybir
import concourse.tile as tile
from concourse.bass_utils import run_bass_kernel_spmd

N_NODES = 100000
E_EDGES = 1600000
D = 128
N_CORES = 8
NPC = N_NODES // N_CORES          # 12500 nodes per core
P = 128
NBLK = (NPC + P - 1) // P         # 98 dst blocks per core (last has 84)
NCHUNK = 4
# chunk boundaries over the h table; each chunk < 32768 rows (int16 gather idx)
CBB = np.array([0, 28500, 57000, 85500, 100000], dtype=np.int64)
G = 7                             # dst blocks per supergroup (one gather group)
NSG = NBLK // G                   # 14 supergroups

FP16 = np.float16
USE_SHARED = True
GMAX = 1024  # max idxs per dma_gather (single-packet: 64 descs x 16 engines)


def _prep(edge_src, edge_dst, edge_weight):
    """Host-side scheduling: bucket edges by (core, dst-block, src-chunk),
    pad each bucket to a multiple of 128 slots (shared across cores), and
    build per-core gather-index arrays plus dense selector matrices."""
    src = edge_src.astype(np.int64)
    dst = edge_dst.astype(np.int64)
    ew = edge_weight.astype(np.float32)

    core = dst // NPC
    ldst = dst - core * NPC
    blk = ldst // P                       # 0..NBLK-1
    doff = (ldst % P).astype(np.int64)
    chunk = np.searchsorted(CBB, src, side="right") - 1
    lsrc = (src - CBB[chunk]).astype(np.int16)

    nseg = N_CORES * NBLK * NCHUNK
    seg = (core * NBLK + blk) * NCHUNK + chunk
    counts = np.bincount(seg, minlength=nseg).reshape(N_CORES, NBLK, NCHUNK)

    # static slot count per (block, chunk): max over cores, rounded to 128
    S = ((counts.max(axis=0) + P - 1) // P) * P      # [NBLK, NCHUNK]
    Ssg = S.reshape(NSG, G, NCHUNK)
    L = Ssg.sum(axis=1)                              # [NSG, NCHUNK] gather sizes
    SLOTS_G = L.sum(axis=1)                          # [NSG] slots per supergroup

    # offsets
    REG_OFF = np.zeros((NSG, NCHUNK), dtype=np.int64)    # msg-local region start
    REG_OFF[:, 1:] = np.cumsum(L, axis=1)[:, :-1]
    # SUB_OFF[b, ch]: msg-local slot offset of block b's bucket inside its sg
    sub = np.cumsum(Ssg, axis=1)                        # cumsum over blocks in sg
    SUB_OFF = np.zeros((NSG, G, NCHUNK), dtype=np.int64)
    SUB_OFF[:, 1:, :] = sub[:, :-1, :]
    SUB_OFF = SUB_OFF + REG_OFF[:, None, :]
    # IDX_BASE[g, ch]: start of gather (g, ch)'s idx list in the flat idx array
    Lflat = L.reshape(-1)
    IDX_BASE = np.zeros(NSG * NCHUNK, dtype=np.int64)
    IDX_BASE[1:] = np.cumsum(Lflat)[:-1]
    IDX_BASE = IDX_BASE.reshape(NSG, NCHUNK)
    TOTIDX = int(Lflat.sum())
    BAT_BASE = np.zeros(NSG, dtype=np.int64)
    BAT_BASE[1:] = np.cumsum(SLOTS_G // P)[:-1]
    NBAT = int((SLOTS_G // P).sum())

    # rank of each edge within its (core, blk, chunk) bucket; within a
    # bucket, slots are ordered by source row so the gather descriptors walk
    # ascending HBM addresses (sel absorbs the slot->dst mapping)
    order = np.lexsort((lsrc, seg))
    counts_flat = counts.reshape(-1)
    starts = np.zeros(nseg, dtype=np.int64)
    starts[1:] = np.cumsum(counts_flat)[:-1]
    rank_sorted = np.arange(E_EDGES, dtype=np.int64) - starts[seg[order]]
    rank = np.empty(E_EDGES, dtype=np.int64)
    rank[order] = rank_sorted

    g_of = blk // G
    # position inside the (g, chunk) gather idx list
    within = (SUB_OFF[g_of, blk % G, chunk] - REG_OFF[g_of, chunk]) + rank
    idxpos = IDX_BASE[g_of, chunk] + within
    # msg-local slot inside the supergroup
    slot = SUB_OFF[g_of, blk % G, chunk] + rank
    bat = BAT_BASE[g_of] + slot // P
    part = slot % P

    idx_arr = np.zeros((N_CORES, TOTIDX), dtype=np.int16)
    idx_arr[core, idxpos] = lsrc
    # dense selector matrices: sel[part, bat*128 + d] = ew * (d == doff)
    sel_arr = np.zeros((N_CORES, P, NBAT * P), dtype=FP16)
    sel_arr[core, part, bat * P + doff] = ew.astype(FP16)

    # wrap idx arrays in 16 partitions, replicate to 128 (Q7 queue groups)
    idx_wrapped = np.ascontiguousarray(
        np.tile(idx_arr.reshape(N_CORES, -1, 16).transpose(0, 2, 1), (1, 8, 1))
    )  # [cores, 128, TOTIDX//16]

    meta = {
        "S": S, "L": L, "SLOTS_G": SLOTS_G, "REG_OFF": REG_OFF,
        "SUB_OFF": SUB_OFF, "IDX_BASE": IDX_BASE, "BAT_BASE": BAT_BASE,
        "TOTIDX": TOTIDX, "NBAT": NBAT,
    }
    return meta, idx_wrapped, sel_arr


def _build(meta):
    S = meta["S"]; L = meta["L"]; SLOTS_G = meta["SLOTS_G"]
    REG_OFF = meta["REG_OFF"]; SUB_OFF = meta["SUB_OFF"]
    IDX_BASE = meta["IDX_BASE"]; BAT_BASE = meta["BAT_BASE"]
    TOTIDX = meta["TOTIDX"]; NBAT = meta["NBAT"]
    NPAD = NBLK * P

    nc = bacc.Bacc("TRN2", target_bir_lowering=False, debug=False,
                   num_devices=N_CORES, num_swdge_queues=4)
    xT_in = nc.dram_tensor("xT_in", [P, NPAD], mybir.dt.float16,
                           kind="ExternalInput")
    w1_in = nc.dram_tensor("w1_in", [P, D], mybir.dt.float16, kind="ExternalInput")
    w2_in = nc.dram_tensor("w2_in", [P, D], mybir.dt.float16, kind="ExternalInput")
    idx_in = nc.dram_tensor("idx_in", [P, TOTIDX // 16], mybir.dt.int16,
                            kind="ExternalInput")
    sel_in = nc.dram_tensor("sel_in", [P, NBAT * P], mybir.dt.float16,
                            kind="ExternalInput")
    out_dram = nc.dram_tensor("out", [NPC, D], mybir.dt.float32,
                              kind="ExternalOutput")

    with tile.TileContext(nc) as tc:
        with tc.tile_pool(name="const", bufs=1) as const, \
             tc.tile_pool(name="msg", bufs=3) as msgp, \
             tc.tile_pool(name="sel", bufs=2) as selp, \
             tc.tile_pool(name="idxp", bufs=3) as idxp, \
             tc.tile_pool(name="xblk", bufs=3) as xbp, \
             tc.tile_pool(name="hcast", bufs=4) as hcp, \
             tc.tile_pool(name="outp", bufs=4) as outp, \
             tc.tile_pool(name="psA", bufs=6, space="PSUM") as psA, \
             tc.tile_pool(name="psD", bufs=2, space="PSUM") as psD, \
             tc.tile_pool(name="dram", bufs=1, space="DRAM") as dram:

            # ---- resident constants ----
            w1_t = const.tile([P, D], mybir.dt.float16)
            nc.sync.dma_start(out=w1_t[:], in_=w1_in[:])
            w2_t = const.tile([P, D], mybir.dt.float16)
            nc.sync.dma_start(out=w2_t[:], in_=w2_in[:])
            x2T = const.tile([P, NPAD], mybir.dt.float16)

            # ---- DRAM internals ----
            h_bounce = [dram.tile([NPC, D], mybir.dt.float16, tag=f"hb{i}",
                                  name=f"h_bounce{i}") for i in range(2)]
            h_full = [dram.tile([N_NODES, D], mybir.dt.float16, tag=f"hf{i}",
                                addr_space=("Shared" if USE_SHARED else
                                            "Local"),
                                name=f"h_full{i}") for i in range(2)]

            def dense_blocks(w_tile, t0, t1, from_dram, bounce):
                xb = None
                for t in range(t0, t1):
                    if from_dram:
                        if xb is None or t % 4 == 0:
                            nb = min(4, NBLK - t)
                            xb = xbp.tile([P, nb * P], mybir.dt.float16,
                                          tag="xblk")
                            xb0 = t
                            nc.sync.dma_start(
                                out=xb[:], in_=xT_in[:, t * P:(t + nb) * P])
                        lhsT = xb[:, (t - xb0) * P:(t - xb0 + 1) * P]
                    else:
                        lhsT = x2T[:, t * P:(t + 1) * P]
                    ps = psD.tile([P, D], mybir.dt.float32, tag="dense")
                    nc.tensor.matmul(out=ps[:], lhsT=lhsT, rhs=w_tile[:],
                                     start=True, stop=True)
                    hb = hcp.tile([P, D], mybir.dt.float16, tag="hcast")
                    nc.scalar.activation(out=hb[:], in_=ps[:],
                                         func=mybir.ActivationFunctionType.Copy)
                    rows = min(P, NPC - t * P)
                    nc.sync.dma_start(out=bounce[t * P:t * P + rows, :],
                                      in_=hb[:rows, :])

            def allgather(i):
                nc.gpsimd.collective_compute(
                    "AllGather", mybir.AluOpType.bypass,
                    replica_groups=[list(range(N_CORES))],
                    ins=[h_bounce[i].opt()], outs=[h_full[i].opt()],
                )

            cnt_regs = {}

            def cnt_reg(cnt):
                if cnt not in cnt_regs:
                    cnt_regs[cnt] = nc.gpsimd.to_reg(cnt)
                return cnt_regs[cnt]

            def aggregate(layer, full, sg_tail=None):
                qi = 0
                for g in range(NSG):
                    nslot = int(SLOTS_G[g])
                    msg = msgp.tile([P, nslot // P, D], mybir.dt.float16,
                                    tag="msg")
                    gi0 = int(IDX_BASE[g, 0]) // 16
                    idx_t = idxp.tile([P, nslot // 16], mybir.dt.int16,
                                      tag="idx")
                    nc.scalar.dma_start(out=idx_t[:],
                                        in_=idx_in[:, gi0:gi0 + nslot // 16])
                    for ch in range(NCHUNK):
                        lg = int(L[g, ch])
                        if lg == 0:
                            continue
                        # single_packet dma_gather caps at 64 descs/engine
                        # (one 16 KiB packet) = 1024 indices per instruction
                        for p0 in range(0, lg, GMAX):
                            cnt = min(GMAX, lg - p0)
                            r0 = (int(REG_OFF[g, ch]) + p0) // P
                            i0 = (int(IDX_BASE[g, ch]) + p0) // 16
                            nc.gpsimd.dma_gather(
                                out_ap=msg[:, r0:r0 + cnt // P, :],
                                in_ap=full[int(CBB[ch]):int(CBB[ch + 1]), :],
                                idxs_ap=idx_t[:, i0 - gi0:i0 - gi0 + cnt // 16],
                                num_idxs=cnt, num_idxs_reg=cnt_reg(cnt), elem_size=D,
                                queue_num=qi % 4,
                            )
                            qi += 1
                    sel_sg = selp.tile([P, nslot], mybir.dt.float16, tag="sel")
                    c0 = int(BAT_BASE[g]) * P
                    half = (nslot // 2 // P) * P
                    nc.sync.dma_start(out=sel_sg[:, :half],
                                      in_=sel_in[:, c0:c0 + half])
                    nc.sync.dma_start(out=sel_sg[:, half:],
                                      in_=sel_in[:, c0 + half:c0 + nslot])
                    for j in range(G):
                        b = g * G + j
                        batches = []
                        for ch in range(NCHUNK):
                            nb = int(S[b, ch]) // P
                            s0 = int(SUB_OFF[g, j, ch]) // P
                            batches += [s0 + k for k in range(nb)]
                        rows = min(P, NPC - b * P)
                        ps = psA.tile([P, D], mybir.dt.float32, tag="agg")
                        for i, s in enumerate(batches):
                            first, last = (i == 0), (i == len(batches) - 1)
                            sl = sel_sg[:, s * P:(s + 1) * P]
                            if layer == 1:
                                # psum[feat, dst] = sum_e msg[e, f] sel[e, d]
                                nc.tensor.matmul(out=ps[:], lhsT=msg[:, s, :],
                                                 rhs=sl, start=first,
                                                 stop=last)
                            else:
                                # psum[dst, feat]
                                nc.tensor.matmul(out=ps[:], lhsT=sl,
                                                 rhs=msg[:, s, :], start=first,
                                                 stop=last)
                        if layer == 1:
                            nc.scalar.activation(
                                out=x2T[:, b * P:(b + 1) * P], in_=ps[:],
                                func=mybir.ActivationFunctionType.Tanh)
                        else:
                            ot = outp.tile([P, D], mybir.dt.float32, tag="out")
                            nc.scalar.activation(
                                out=ot[:], in_=ps[:],
                                func=mybir.ActivationFunctionType.Tanh)
                            nc.sync.dma_start(out=out_dram[b * P:b * P + rows, :],
                                              in_=ot[:rows, :])
                    if sg_tail is not None:
                        sg_tail(g)

            dense_blocks(w1_t, 0, NBLK, from_dram=True, bounce=h_bounce[0])
            allgather(0)
            aggregate(layer=1, full=h_full[0],
                      sg_tail=lambda g: dense_blocks(w2_t, g * G, (g + 1) * G,
                                                     from_dram=False,
                                                     bounce=h_bounce[1]))
            allgather(1)
            aggregate(layer=2, full=h_full[1])

    nc.compile()
    return nc


def kernel(x, edge_src, edge_dst, edge_weight, W1, W2, _trace=False):
    assert x.shape == (N_NODES, D) and edge_src.shape == (E_EDGES,)
    meta, idx_w, sel_arr = _prep(edge_src, edge_dst, edge_weight)
    nc = _build(meta)

    NPAD = NBLK * P
    w1 = np.ascontiguousarray(W1.astype(FP16))
    w2 = np.ascontiguousarray(W2.astype(FP16))
    in_maps = []
    for c in range(N_CORES):
        xT = np.zeros((P, NPAD), dtype=FP16)
        xT[:, :NPC] = x[c * NPC:(c + 1) * NPC].T.astype(FP16)
        in_maps.append({
            "xT_in": xT,
            "w1_in": w1, "w2_in": w2,
            "idx_in": idx_w[c],
            "sel_in": sel_arr[c],
        })
    res = run_bass_kernel_spmd(nc, in_maps, core_ids=list(range(N_CORES)),
                               trace=_trace)
    out = np.concatenate([res.results[c]["out"] for c in range(N_CORES)], axis=0)
    if _trace:
        kernel.last_results = res
    return out


# revision 23
# speedup vs baseline: 1.0487x; 1.0487x over previous
"""Two-layer GraphConv (GCN) on 8 Trainium2 NeuronCores.

Reference computation (per layer):
    h   = x @ W                      [N, D]
    msg = h[edge_src] * edge_weight  [E, D]
    out = segment_sum(msg, edge_dst) [N, D]
    x'  = tanh(out)

Distribution strategy (from the sharding hint): partition nodes across the 8
cores by destination range (12500 nodes/core). Each core:
  - computes h for its own node slice (dense matmul, fp16),
  - AllGathers the full h table into DRAM,
  - gathers h[src] rows for its own (dst-sorted) edges with dma_gather,
  - aggregates per 128-dst block with a one-hot selector matmul into PSUM
    (sel[e, d] = ew[e] * (d == dst_off[e])); the selector matrices are
    precomputed on the host and streamed from DRAM on the HWDGE path, so no
    on-chip engine spends time building them,
  - applies tanh on the Scalar engine.

Layer 1 produces its output transposed ([feat, node]) so that layer 2's dense
matmul needs no runtime transpose; layer 2 produces [node, feat] directly for
the final output.

dma_gather uses int16 indices, so the h table is addressed in 4 row-chunks
(< 32768 rows each). Edges are bucketed by (dst block, src chunk); each
bucket is padded to a multiple of 128 slots with ew=0 entries so the matmul
schedule is static and identical across cores (SPMD: one instruction stream).
The schedule constants are derived from the actual edge data at call time,
then compiled fresh — so the kernel is correct for any input values.
"""

import numpy as np

import concourse.bacc as bacc
import concourse.mybir as mybir
import concourse.tile as tile
from concourse.bass_utils import run_bass_kernel_spmd

N_NODES = 100000
E_EDGES = 1600000
D = 128
N_CORES = 8
NPC = N_NODES // N_CORES          # 12500 nodes per core
P = 128
NBLK = (NPC + P - 1) // P         # 98 dst blocks per core (last has 84)
NCHUNK = 4
# chunk boundaries over the h table; each chunk < 32768 rows (int16 gather idx)
CBB = np.array([0, 28500, 57000, 85500, 100000], dtype=np.int64)
G = 7                             # dst blocks per supergroup (one gather group)
NSG = NBLK // G                   # 14 supergroups

FP16 = np.float16
USE_SHARED = True
GMAX = 1024  # max idxs per dma_gather (single-packet: 64 descs x 16 engines)


def _prep(edge_src, edge_dst, edge_weight):
    """Host-side scheduling: bucket edges by (core, dst-block, src-chunk),
    pad each bucket to a multiple of 128 slots (shared across cores), and
    build per-core gather-index arrays plus dense selector matrices."""
    src = edge_src.astype(np.int64)
    dst = edge_dst.astype(np.int64)
    ew = edge_weight.astype(np.float32)

    core = dst // NPC
    ldst = dst - core * NPC
    blk = ldst // P                       # 0..NBLK-1
    doff = (ldst % P).astype(np.int64)
    chunk = np.searchsorted(CBB, src, side="right") - 1
    lsrc = (src - CBB[chunk]).astype(np.int16)

    nseg = N_CORES * NBLK * NCHUNK
    seg = (core * NBLK + blk) * NCHUNK + chunk
    counts = np.bincount(seg, minlength=nseg).reshape(N_CORES, NBLK, NCHUNK)

    # static slot count per (block, chunk): max over cores, rounded to 128
    S = ((counts.max(axis=0) + P - 1) // P) * P      # [NBLK, NCHUNK]
    Ssg = S.reshape(NSG, G, NCHUNK)
    L = Ssg.sum(axis=1)                              # [NSG, NCHUNK] gather sizes
    SLOTS_G = L.sum(axis=1)                          # [NSG] slots per supergroup

    # offsets
    REG_OFF = np.zeros((NSG, NCHUNK), dtype=np.int64)    # msg-local region start
    REG_OFF[:, 1:] = np.cumsum(L, axis=1)[:, :-1]
    # SUB_OFF[b, ch]: msg-local slot offset of block b's bucket inside its sg
    sub = np.cumsum(Ssg, axis=1)                        # cumsum over blocks in sg
    SUB_OFF = np.zeros((NSG, G, NCHUNK), dtype=np.int64)
    SUB_OFF[:, 1:, :] = sub[:, :-1, :]
    SUB_OFF = SUB_OFF + REG_OFF[:, None, :]
    # IDX_BASE[g, ch]: start of gather (g, ch)'s idx list in the flat idx array
    Lflat = L.reshape(-1)
    IDX_BASE = np.zeros(NSG * NCHUNK, dtype=np.int64)
    IDX_BASE[1:] = np.cumsum(Lflat)[:-1]
    IDX_BASE = IDX_BASE.reshape(NSG, NCHUNK)
    TOTIDX = int(Lflat.sum())
    BAT_BASE = np.zeros(NSG, dtype=np.int64)
    BAT_BASE[1:] = np.cumsum(SLOTS_G // P)[:-1]
    NBAT = int((SLOTS_G // P).sum())

    # rank of each edge within its (core, blk, chunk) bucket; within a
    # bucket, slots are ordered by source row so the gather descriptors walk
    # ascending HBM addresses (sel absorbs the slot->dst mapping)
    order = np.lexsort((lsrc, seg))
    counts_flat = counts.reshape(-1)
    starts = np.zeros(nseg, dtype=np.int64)
    starts[1:] = np.cumsum(counts_flat)[:-1]
    rank_sorted = np.arange(E_EDGES, dtype=np.int64) - starts[seg[order]]
    rank = np.empty(E_EDGES, dtype=np.int64)
    rank[order] = rank_sorted

    g_of = blk // G
    # position inside the (g, chunk) gather idx list
    within = (SUB_OFF[g_of, blk % G, chunk] - REG_OFF[g_of, chunk]) + rank
    idxpos = IDX_BASE[g_of, chunk] + within
    # msg-local slot inside the supergroup
    slot = SUB_OFF[g_of, blk % G, chunk] + rank
    bat = BAT_BASE[g_of] + slot // P
    part = slot % P

    idx_arr = np.zeros((N_CORES, TOTIDX), dtype=np.int16)
    idx_arr[core, idxpos] = lsrc
    # dense selector matrices: sel[part, bat*128 + d] = ew * (d == doff)
    sel_arr = np.zeros((N_CORES, P, NBAT * P), dtype=FP16)
    sel_arr[core, part, bat * P + doff] = ew.astype(FP16)

    # wrap idx arrays in 16 partitions, replicate to 128 (Q7 queue groups)
    idx_wrapped = np.ascontiguousarray(
        np.tile(idx_arr.reshape(N_CORES, -1, 16).transpose(0, 2, 1), (1, 8, 1))
    )  # [cores, 128, TOTIDX//16]

    meta = {
        "S": S, "L": L, "SLOTS_G": SLOTS_G, "REG_OFF": REG_OFF,
        "SUB_OFF": SUB_OFF, "IDX_BASE": IDX_BASE, "BAT_BASE": BAT_BASE,
        "TOTIDX": TOTIDX, "NBAT": NBAT,
    }
    return meta, idx_wrapped, sel_arr


def _build(meta):
    S = meta["S"]; L = meta["L"]; SLOTS_G = meta["SLOTS_G"]
    REG_OFF = meta["REG_OFF"]; SUB_OFF = meta["SUB_OFF"]
    IDX_BASE = meta["IDX_BASE"]; BAT_BASE = meta["BAT_BASE"]
    TOTIDX = meta["TOTIDX"]; NBAT = meta["NBAT"]
    NPAD = NBLK * P

    nc = bacc.Bacc("TRN2", target_bir_lowering=False, debug=False,
                   num_devices=N_CORES, num_swdge_queues=4)
    xT_in = nc.dram_tensor("xT_in", [P, NPAD], mybir.dt.float16,
                           kind="ExternalInput")
    w1_in = nc.dram_tensor("w1_in", [P, D], mybir.dt.float16, kind="ExternalInput")
    w2_in = nc.dram_tensor("w2_in", [P, D], mybir.dt.float16, kind="ExternalInput")
    idx_in = nc.dram_tensor("idx_in", [P, TOTIDX // 16], mybir.dt.int16,
                            kind="ExternalInput")
    sel_in = nc.dram_tensor("sel_in", [P, NBAT * P], mybir.dt.float16,
                            kind="ExternalInput")
    out_dram = nc.dram_tensor("out", [NPC, D], mybir.dt.float32,
                              kind="ExternalOutput")

    with tile.TileContext(nc) as tc:
        with tc.tile_pool(name="const", bufs=1) as const, \
             tc.tile_pool(name="msg", bufs=3) as msgp, \
             tc.tile_pool(name="sel", bufs=2) as selp, \
             tc.tile_pool(name="idxp", bufs=3) as idxp, \
             tc.tile_pool(name="xblk", bufs=3) as xbp, \
             tc.tile_pool(name="hcast", bufs=4) as hcp, \
             tc.tile_pool(name="outp", bufs=4) as outp, \
             tc.tile_pool(name="psA", bufs=6, space="PSUM") as psA, \
             tc.tile_pool(name="psD", bufs=2, space="PSUM") as psD, \
             tc.tile_pool(name="dram", bufs=1, space="DRAM") as dram:

            # ---- resident constants ----
            w1_t = const.tile([P, D], mybir.dt.float16)
            nc.sync.dma_start(out=w1_t[:], in_=w1_in[:])
            w2_t = const.tile([P, D], mybir.dt.float16)
            nc.sync.dma_start(out=w2_t[:], in_=w2_in[:])
            x2T = const.tile([P, NPAD], mybir.dt.float16)

            # ---- DRAM internals ----
            h_bounce = [dram.tile([NPC, D], mybir.dt.float16, tag=f"hb{i}",
                                  name=f"h_bounce{i}") for i in range(2)]
            h_full = [dram.tile([N_NODES, D], mybir.dt.float16, tag=f"hf{i}",
                                addr_space=("Shared" if USE_SHARED else
                                            "Local"),
                                name=f"h_full{i}") for i in range(2)]

            def dense_blocks(w_tile, t0, t1, from_dram, bounce):
                xb = None
                for t in range(t0, t1):
                    if from_dram:
                        if xb is None or t % 4 == 0:
                            nb = min(4, NBLK - t)
                            xb = xbp.tile([P, nb * P], mybir.dt.float16,
                                          tag="xblk")
                            xb0 = t
                            nc.sync.dma_start(
                                out=xb[:], in_=xT_in[:, t * P:(t + nb) * P])
                        lhsT = xb[:, (t - xb0) * P:(t - xb0 + 1) * P]
                    else:
                        lhsT = x2T[:, t * P:(t + 1) * P]
                    ps = psD.tile([P, D], mybir.dt.float32, tag="dense")
                    nc.tensor.matmul(out=ps[:], lhsT=lhsT, rhs=w_tile[:],
                                     start=True, stop=True)
                    hb = hcp.tile([P, D], mybir.dt.float16, tag="hcast")
                    nc.scalar.activation(out=hb[:], in_=ps[:],
                                         func=mybir.ActivationFunctionType.Copy)
                    rows = min(P, NPC - t * P)
                    nc.sync.dma_start(out=bounce[t * P:t * P + rows, :],
                                      in_=hb[:rows, :])

            def allgather(i):
                nc.gpsimd.collective_compute(
                    "AllGather", mybir.AluOpType.bypass,
                    replica_groups=[list(range(N_CORES))],
                    ins=[h_bounce[i].opt()], outs=[h_full[i].opt()],
                )

            cnt_regs = {}

            def cnt_reg(cnt):
                if cnt not in cnt_regs:
                    cnt_regs[cnt] = nc.gpsimd.to_reg(cnt)
                return cnt_regs[cnt]

            def aggregate(layer, full, sg_tail=None):
                qi = 0
                for g in range(NSG):
                    nslot = int(SLOTS_G[g])
                    msg = msgp.tile([P, nslot // P, D], mybir.dt.float16,
                                    tag="msg")
                    gi0 = int(IDX_BASE[g, 0]) // 16
                    idx_t = idxp.tile([P, nslot // 16], mybir.dt.int16,
                                      tag="idx")
                    nc.scalar.dma_start(out=idx_t[:],
                                        in_=idx_in[:, gi0:gi0 + nslot // 16])
                    for ch in range(NCHUNK):
                        lg = int(L[g, ch])
                        if lg == 0:
                            continue
                        # single_packet dma_gather caps at 64 descs/engine
                        # (one 16 KiB packet) = 1024 indices per instruction
                        for p0 in range(0, lg, GMAX):
                            cnt = min(GMAX, lg - p0)
                            r0 = (int(REG_OFF[g, ch]) + p0) // P
                            i0 = (int(IDX_BASE[g, ch]) + p0) // 16
                            nc.gpsimd.dma_gather(
                                out_ap=msg[:, r0:r0 + cnt // P, :],
                                in_ap=full[int(CBB[ch]):int(CBB[ch + 1]), :],
                                idxs_ap=idx_t[:, i0 - gi0:i0 - gi0 + cnt // 16],
                                num_idxs=cnt, num_idxs_reg=cnt_reg(cnt), elem_size=D,
                                queue_num=qi % 4,
                            )
                            qi += 1
                    sel_sg = selp.tile([P, nslot], mybir.dt.float16, tag="sel")
                    c0 = int(BAT_BASE[g]) * P
                    nc.sync.dma_start(out=sel_sg[:],
                                      in_=sel_in[:, c0:c0 + nslot])
                    for j in range(G):
                        b = g * G + j
                        batches = []
                        for ch in range(NCHUNK):
                            nb = int(S[b, ch]) // P
                            s0 = int(SUB_OFF[g, j, ch]) // P
                            batches += [s0 + k for k in range(nb)]
                        rows = min(P, NPC - b * P)
                        ps = psA.tile([P, D], mybir.dt.float32, tag="agg")
                        for i, s in enumerate(batches):
                            first, last = (i == 0), (i == len(batches) - 1)
                            sl = sel_sg[:, s * P:(s + 1) * P]
                            if layer == 1:
                                # psum[feat, dst] = sum_e msg[e, f] sel[e, d]
                                nc.tensor.matmul(out=ps[:], lhsT=msg[:, s, :],
                                                 rhs=sl, start=first,
                                                 stop=last)
                            else:
                                # psum[dst, feat]
                                nc.tensor.matmul(out=ps[:], lhsT=sl,
                                                 rhs=msg[:, s, :], start=first,
                                                 stop=last)
                        if layer == 1:
                            nc.scalar.activation(
                                out=x2T[:, b * P:(b + 1) * P], in_=ps[:],
                                func=mybir.ActivationFunctionType.Tanh)
                        else:
                            ot = outp.tile([P, D], mybir.dt.float32, tag="out")
                            nc.scalar.activation(
                                out=ot[:], in_=ps[:],
                                func=mybir.ActivationFunctionType.Tanh)
                            nc.sync.dma_start(out=out_dram[b * P:b * P + rows, :],
                                              in_=ot[:rows, :])
                    if sg_tail is not None:
                        sg_tail(g)

            dense_blocks(w1_t, 0, NBLK, from_dram=True, bounce=h_bounce[0])
            allgather(0)
            aggregate(layer=1, full=h_full[0],
                      sg_tail=lambda g: dense_blocks(w2_t, g * G, (g + 1) * G,
                                                     from_dram=False,
                                                     bounce=h_bounce[1]))
            allgather(1)
            aggregate(layer=2, full=h_full[1])

    nc.compile()
    return nc


def kernel(x, edge_src, edge_dst, edge_weight, W1, W2, _trace=False):
    assert x.shape == (N_NODES, D) and edge_src.shape == (E_EDGES,)
    meta, idx_w, sel_arr = _prep(edge_src, edge_dst, edge_weight)
    nc = _build(meta)

    NPAD = NBLK * P
    w1 = np.ascontiguousarray(W1.astype(FP16))
    w2 = np.ascontiguousarray(W2.astype(FP16))
    in_maps = []
    for c in range(N_CORES):
        xT = np.zeros((P, NPAD), dtype=FP16)
        xT[:, :NPC] = x[c * NPC:(c + 1) * NPC].T.astype(FP16)
        in_maps.append({
            "xT_in": xT,
            "w1_in": w1, "w2_in": w2,
            "idx_in": idx_w[c],
            "sel_in": sel_arr[c],
        })
    res = run_bass_kernel_spmd(nc, in_maps, core_ids=list(range(N_CORES)),
                               trace=_trace)
    out = np.concatenate([res.results[c]["out"] for c in range(N_CORES)], axis=0)
    if _trace:
        kernel.last_results = res
    return out
